# revision 23
# baseline (speedup 1.0000x reference)
"""Trainium2 Bass kernel v3 for nn_PlatonicConv (linear-attention GNN message passing).

Math (reference):
  q = rope(x@Wq + bq, phase);  k = rope(ones, phase);  v = x@Wv + bv
  KV_b[g] = (1/AVG) * sum_{n in graph b} k[n,g,:] (x) v[n,g,:]
  out[n]  = concat_g( q'[n,g,:] @ KV_b[g] ) @ Wo + bo

Device formulation (per core, 8 graphs/core, slot-padded columns):
  - host precomputes trig (feature-major cos/sin planes) and
    k = rope(ones) node-major with A-ORDERED columns.
  - A-order over rope pairs t = g*16+p (3 tiles of 128):
      rows   0:128 = E_t, t=0..127      (q0)
      rows 128:256 = O_t, t=0..127      (q1)
      rows 256:384 = E_t t=128..191 (64) | O_t t=128..191 (64)   (q2n)
  - x and trig ship together: one DRAM buffer [128, 7, NP] whose
    planes are 0:3 = x^T blocks, 3 = cos_low, 4 = sin_low,
    5 = [cos_high; sin_high], 6 = [sin_high; cos_high]. One DMA per
    chunk covers both the projections' input and the rope trig.
  - rope runs entirely on DVE with independent temps (no WAR chains);
    the 64-row q2 ops use plane 5/6 halves so every op has uniform
    input base partitions (out-shift only, as supported).
  - KV^T computed per graph into 2 psum regions with dense 4-group
    blocks; evacuated via block-diag MASK multiply (1 vector op each);
    Mb = KV^T-masked @ Wo via 5 contiguous-stationary matmuls.
  - out[n] = q'[n] @ M_{b(n)}; psum evacuated as bf16, DMA'd out.
  - startup: dummy 1x16 matmuls warm the PE (HAM un-throttle) while
    the first chunk's DMA lands; DMA issue order prioritizes
    wq -> xt(chunk0) -> wv -> trig(chunk0) so the pipeline fills fast.

Self-contained: hardcodes shapes; shards/pads on host inside kernel().
"""

import math
import os
from contextlib import ExitStack

import ml_dtypes
import numpy as np

import concourse.bacc as bacc_mod
import concourse.bass as bass
import concourse.mybir as mybir
import concourse.tile as tile
from concourse.bass_utils import run_bass_kernel_spmd


def _ensure_ntff_hook():
    """Register the axon NTFF profile hook if the image's antenv lacks it."""
    try:
        import antenv.axon_hooks  # noqa: F401

        return True
    except ImportError:
        pass
    try:
        import sys
        import types

        import antenv
        from trn_agent_boot.trn_boot import _ntff_profile_via_ctypes

        mod = types.ModuleType("antenv.axon_hooks")
        _hook = [None]
        mod.set_axon_ntff_profile_hook = lambda h: _hook.__setitem__(0, h)
        mod.get_axon_ntff_profile_hook = lambda: _hook[0]
        sys.modules["antenv.axon_hooks"] = mod
        antenv.axon_hooks = mod
        mod.set_axon_ntff_profile_hook(
            _ntff_profile_via_ctypes("/opt/axon/libaxon_pjrt.so")
        )
        return True
    except Exception:
        return False


FP32 = mybir.dt.float32
BF16 = mybir.dt.bfloat16
AF = mybir.ActivationFunctionType

N = 32768
C = 384
E = 384
G = 12
D = 32
P = 16
SD = 3
NUM_GRAPHS = 64
NCORES = 8
GPD = NUM_GRAPHS // NCORES
AVG = float(N) / NUM_GRAPHS
NT = 192  # rope pairs = G*P
W = 512  # streaming window
NWARM = int(os.environ.get("PLATCONV_NWARM", "64"))


def _a_order_cols():
    """perm such that A-order column r is original q-dim perm[r]."""
    perm = np.empty(E, dtype=np.int64)
    for r in range(E):
        if r < 128:
            t, odd = r, 0
        elif r < 256:
            t, odd = r - 128, 1
        elif r < 320:
            t, odd = 128 + (r - 256), 0
        else:
            t, odd = 128 + (r - 320), 1
        perm[r] = (t // 16) * 32 + 2 * (t % 16) + odd
    return perm


_APERM = _a_order_cols()


def _kv_mask():
    """[128, 640] bf16 mask selecting valid diag blocks of the KV psum layout.

    cols 0:256   = (e-chunk0, kA 0:256: E-main|O-main) -> g 0..3 rows
    cols 256:512 = (e-chunk1, kA 0:256)               -> g 4..7 rows
    cols 512:640 = (e-chunk2, kA 256:384: tail E|O)   -> g 8..11 rows
    """
    m = np.zeros((128, 640), dtype=np.float32)
    for ech in range(3):
        base = 256 * ech
        ncols = 256 if ech < 3 - 1 else 128
        for cc in range(ncols):
            if ech < 2:
                t = cc % 128  # E-main or O-main: t = 0..127
                g = t // 16
                ok = g - 4 * ech
            else:
                t = 128 + (cc % 64)  # tail: t = 128..191
                g = t // 16
                ok = g - 8
            if 0 <= ok < 4:
                m[32 * ok : 32 * ok + 32, base + cc] = 1.0
    return m


_CACHE = {}


def _build(slots: tuple, has_bias: bool):
    key = (slots, has_bias)
    if key in _CACHE:
        return _CACHE[key]

    assert len(slots) == GPD
    soff = [0]
    for s_ in slots:
        soff.append(soff[-1] + s_)
    NP = (soff[-1] + 127) // 128 * 128
    NTILE = NP // 128
    NCH = (NP + W - 1) // W
    # per-graph covering k-tile ranges (boundary tiles duplicated, host
    # masks each copy to its own graph's nodes)
    kti = [0]
    for j in range(GPD):
        cov = (soff[j] % 128 + slots[j] + 127) // 128
        kti.append(kti[-1] + cov)
    KTOT = kti[-1]

    nc = bacc_mod.Bacc()

    assert not has_bias, "packed-DMA path assumes zero biases"
    # planes 0:3 = x^T blocks; 3=cos_lo; 4=sin_lo; 5=[cos_hi;sin_hi];
    # 6=[sin_hi;cos_hi]
    xtt_d = nc.declare_dram_parameter("xtt", [128, 7, NP], BF16, isOutput=False)
    kn_d = nc.declare_dram_parameter("kn", [KTOT * 128, E], BF16, isOutput=False)
    # wpk rows: 0:3 wq blocks, 3:6 wv blocks, 6:9 wos blocks
    wpk_d = nc.declare_dram_parameter("wpk", [128, 9, E], BF16, isOutput=False)
    msk_d = nc.declare_dram_parameter("msk", [128, 640], BF16, isOutput=False)
    out_d = nc.declare_dram_parameter("outt", [3, 128, NP], BF16, isOutput=True)

    nk = 3

    with ExitStack() as ctx:
        tc = ctx.enter_context(tile.TileContext(nc))

        consts = ctx.enter_context(tc.tile_pool(name="consts", bufs=1))
        xtp = ctx.enter_context(tc.tile_pool(name="xtp", bufs=3))
        qsb = ctx.enter_context(tc.tile_pool(name="qsb", bufs=2))
        tmp = ctx.enter_context(tc.tile_pool(name="tmp", bufs=2))
        big = ctx.enter_context(tc.tile_pool(name="big", bufs=1))
        kvp = ctx.enter_context(tc.tile_pool(name="kvp", bufs=2))
        mbp = ctx.enter_context(tc.tile_pool(name="mbp", bufs=3))
        outp = ctx.enter_context(tc.tile_pool(name="outp", bufs=2))
        psum = ctx.enter_context(tc.tile_pool(name="psum", bufs=1, space="PSUM"))

        def pbank(tag):
            return psum.tile([128, W], FP32, tag=tag, name=tag)

        # ---- warmup: ACT table + dummy matmuls to un-throttle the PE ----
        warm = consts.tile([1, 16], BF16, tag="warm")
        nc.vector.memset(warm[:], 0.0)
        nc.scalar.activation(warm[:], warm[:], AF.Copy)
        wps = pbank("T5")
        for _ in range(NWARM):
            nc.tensor.matmul(
                wps[0:1, 0:16], warm[:, 0:1], warm[:, 0:16], start=True, stop=True
            )

        # ---- constants ----
        wpk = consts.tile([128, 9, E], BF16, tag="wpk")
        msk = consts.tile([128, 640], BF16, tag="msk")
        wq_t = [wpk[:, bi, :] for bi in range(3)]
        wv_t = [wpk[:, 3 + bi, :] for bi in range(3)]
        wos_t = [wpk[:, 6 + bi, :] for bi in range(3)]

        # ---- persistent SBUF ----
        q0 = big.tile([128, NP], BF16, tag="q0")
        q1 = big.tile([128, NP], BF16, tag="q1")
        q2n = big.tile([128, NP], BF16, tag="q2n")
        v_sb = big.tile([128, NTILE, E], BF16, tag="v_sb")
        k_sb = big.tile([128, KTOT, E], BF16, tag="k_sb")

        def issue_kn_graph(g):
            """Load masked k covering-tiles for graph g."""
            kview = kn_d[kti[g] * 128 : kti[g + 1] * 128, :].rearrange(
                "(t p) e -> p t e", p=128
            )
            nc.sync.dma_start(k_sb[:, kti[g] : kti[g + 1], :], kview)

        # PSUM plan:
        #  T0,T1,T2: Q proj (per chunk)
        #  T3: V proj even subtiles (cols 0:384) + KV tail region (cols 384:512)
        #  T4: V proj odd subtiles (cols 0:384) + KV main (cols 0:512)
        #  T5,T6,T7: Mb (qch0..2), then out windows rotate T5..T7

        # ------------------------------------------------------------------
        # builders
        # ------------------------------------------------------------------

        def wl(ch):
            return min(W, NP - ch * W)

        def issue_chunk_dma(ch, split):
            n0 = ch * W
            wlen = wl(ch)
            t = xtp.tile([128, 7, W], BF16, tag="xtc", name="xtc")
            if split:
                # per-block DMAs: the first Q matmul only gates on block 0
                for bi in range(3):
                    nc.sync.dma_start(
                        t[:, bi, :wlen], xtt_d[:, bi, n0 : n0 + wlen]
                    )
            else:
                nc.sync.dma_start(t[:, :, :wlen], xtt_d[:, :, n0 : n0 + wlen])
            return t

        def issue_chunk0_trig(t):
            nc.sync.dma_start(t[:, 3:7, : wl(0)], xtt_d[:, 3:7, 0 : wl(0)])

        def issue_q(ch, t, pair):
            """Q projection matmuls + psum evac for chunk ch."""
            wlen = wl(ch)
            qb = [pbank("T0"), pbank("T1"), pbank("T2")]
            for j, ps in enumerate(qb):
                c0 = 128 * j
                for ki in range(nk):
                    nc.tensor.matmul(
                        ps[:, :wlen],
                        wq_t[ki][:, c0 : c0 + 128],
                        t[:, ki, :wlen],
                        start=(ki == 0),
                        stop=(ki == nk - 1),
                    )
            nc.scalar.activation(pair["qe"][:, :wlen], qb[0][:, :wlen], AF.Copy)
            nc.scalar.activation(pair["qo"][:, :wlen], qb[1][:, :wlen], AF.Copy)
            nc.scalar.activation(pair["q2"][:, :wlen], qb[2][:, :wlen], AF.Copy)

        def issue_v(ch, t):
            """V projection: 4 node subtiles, banks T3/T4 alternating."""
            for sub in range(wl(ch) // 128):
                ti = ch * (W // 128) + sub
                f0 = sub * 128
                vps = pbank("T3" if sub % 2 == 0 else "T4")
                for ki in range(nk):
                    nc.tensor.matmul(
                        vps[:, 0:E],
                        t[:, ki, f0 : f0 + 128],
                        wv_t[ki][:],
                        start=(ki == 0),
                        stop=(ki == nk - 1),
                    )
                nc.scalar.activation(v_sb[:, ti, :], vps[:, 0:E], AF.Copy)

        def issue_rope(ch, t, pair):
            """Rope for chunk ch, all on DVE with independent temps."""
            wlen = wl(ch)
            csl = slice(ch * W, ch * W + wlen)
            sl = slice(0, wlen)
            qe, qo, q2 = pair["qe"], pair["qo"], pair["q2"]
            clf = t[:, 3, sl]
            slf = t[:, 4, sl]
            ta0 = tmp.tile([128, W], BF16, tag="ta0", name="ta0")
            tb0 = tmp.tile([128, W], BF16, tag="tb0", name="tb0")
            ta1 = tmp.tile([128, W], BF16, tag="ta1", name="ta1")
            tb1 = tmp.tile([128, W], BF16, tag="tb1", name="tb1")
            tg1 = tmp.tile([64, W], BF16, tag="tg1", name="tg1")
            tg2 = tmp.tile([64, W], BF16, tag="tg2", name="tg2")
            tc2 = tmp.tile([64, W], BF16, tag="tc2", name="tc2")
            td2 = tmp.tile([64, W], BF16, tag="td2", name="td2")
            nc.vector.tensor_mul(ta0[:, sl], qe[:, sl], clf)
            nc.vector.tensor_mul(tb0[:, sl], qo[:, sl], slf)
            nc.vector.tensor_mul(ta1[:, sl], qe[:, sl], slf)
            nc.vector.tensor_mul(tb1[:, sl], qo[:, sl], clf)
            # plane 5 = [cos_hi; sin_hi], plane 6 = [sin_hi; cos_hi]
            nc.vector.tensor_mul(tg1[:, sl], q2[0:64, sl], t[0:64, 5, sl])
            nc.vector.tensor_mul(tg2[:, sl], q2[64:128, sl], t[64:128, 5, sl])
            nc.vector.tensor_mul(tc2[:, sl], q2[0:64, sl], t[0:64, 6, sl])
            nc.vector.tensor_mul(td2[:, sl], q2[64:128, sl], t[64:128, 6, sl])
            nc.vector.tensor_sub(q0[:, csl], ta0[:, sl], tb0[:, sl])
            nc.vector.tensor_add(q1[:, csl], ta1[:, sl], tb1[:, sl])
            nc.vector.tensor_sub(q2n[0:64, csl], tg1[:, sl], tg2[:, sl])
            nc.vector.tensor_add(q2n[64:128, csl], tc2[:, sl], td2[:, sl])

        def issue_kv(j):
            """KV matmuls + mask evac for graph j. Returns kvsb tile."""
            t0 = soff[j] // 128
            TPS = kti[j + 1] - kti[j]
            kvm = pbank("T4")  # main: (e0,kA 0:256) at 0:256, (e1,kA 0:256) at 256:512
            kvt = pbank("T3")  # tail: (e2, kA 256:384) at cols 384:512
            # region-major: start=True clears the whole bank's has_written
            # marks, so each region's accumulation must complete before the
            # next region in the same bank begins.
            kvsb = kvp.tile([128, 640], BF16, tag="kvsb", name="kvsb")
            regions = (
                (kvm[:, 0:256], slice(0, 128), slice(0, 256)),
                (kvm[:, 256:512], slice(128, 256), slice(0, 256)),
                (kvt[:, 384:512], slice(256, 384), slice(256, 384)),
            )
            for dst, vsl, ksl in regions:
                for tt in range(TPS):
                    nc.tensor.matmul(
                        dst,
                        v_sb[:, t0 + tt, vsl],
                        k_sb[:, kti[j] + tt, ksl],
                        start=(tt == 0),
                        stop=(tt == TPS - 1),
                    )
            nc.vector.tensor_mul(kvsb[:, 0:512], kvm[:], msk[:, 0:512])
            nc.vector.tensor_mul(kvsb[:, 512:640], kvt[:, 384:512], msk[:, 512:640])
            return kvsb

        def issue_mb(j, kvsb):
            # Mb: 5 matmuls, contiguous stationaries. Uses the Q banks
            # (T0-2, evacuated early in the chunk) so Mb never waits on
            # the out-window evacuations that own T5-7.
            mb_ps = [pbank("T0"), pbank("T1"), pbank("T2")]
            groups = (
                ((kvsb[:, 0:128], wos_t[0]), (kvsb[:, 256:384], wos_t[1])),
                ((kvsb[:, 128:256], wos_t[0]), (kvsb[:, 384:512], wos_t[1])),
                ((kvsb[:, 512:640], wos_t[2]),),
            )
            mbs = []
            for i, grp in enumerate(groups):
                for gi, (stat, mov) in enumerate(grp):
                    nc.tensor.matmul(
                        mb_ps[i][:, 0:C], stat, mov[:],
                        start=(gi == 0), stop=(gi == len(grp) - 1),
                    )
                mb = mbp.tile([128, C], BF16, tag=f"mb{i}", name=f"mb{i}")
                nc.scalar.activation(mb[:], mb_ps[i][:, 0:C], AF.Copy)
                mbs.append(mb)
            return mbs

        oti = [0]

        def issue_out(j, mbs):
            """Out matmuls + evac + DMA for graph j. Windows split evenly
            (e.g. 572 -> 288+284, not 512+60) to avoid tiny-N matmuls."""
            slot0 = soff[j]
            nwin = (slots[j] + W - 1) // W
            base = slots[j] // nwin // 4 * 4
            wins = []
            o = 0
            for i in range(nwin):
                w = slots[j] - o - base * (nwin - 1 - i)
                wins.append((slot0 + o, w))
                o += w
            for w0, w in wins:
                ost = outp.tile(
                    [128, 3, W], BF16, tag=f"ost{oti[0] % 2}", name=f"ost{oti[0] % 2}"
                )
                for cch in range(3):
                    cc = slice(128 * cch, 128 * (cch + 1))
                    ot = pbank(f"T{5 + cch}")
                    nc.tensor.matmul(
                        ot[:, :w], mbs[0][:, cc], q0[:, w0 : w0 + w],
                        start=True, stop=False,
                    )
                    nc.tensor.matmul(
                        ot[:, :w], mbs[1][:, cc], q1[:, w0 : w0 + w],
                        start=False, stop=False,
                    )
                    nc.tensor.matmul(
                        ot[:, :w], mbs[2][:, cc], q2n[:, w0 : w0 + w],
                        start=False, stop=True,
                    )
                    if cch == 1:
                        nc.scalar.activation(ost[:, cch, :w], ot[:, :w], AF.Copy)
                    else:
                        nc.vector.tensor_copy(ost[:, cch, :w], ot[:, :w])
                nc.sync.dma_start(
                    out_d[:, :, w0 : w0 + w].rearrange("c p w -> p c w"),
                    ost[:, :, :w],
                )
                oti[0] += 1

        # ------------------------------------------------------------------
        # interleaved schedule
        # ------------------------------------------------------------------
        ready_chunk = [
            (soff[g + 1] + W - 1) // W for g in range(GPD)
        ]  # chunks needed before KV_g / OUT_g
        kv_issued = [False] * GPD
        kn_issued = [False] * GPD
        mbs_of = {}
        pending_out = []
        carry_kv = []  # graphs whose KV is deferred to next chunk's stream

        xt_next = None
        for ch in range(NCH):
            pair = {
                k_: qsb.tile([p_, W], BF16, tag=k_, name=k_)
                for k_, p_ in (("qe", 128), ("qo", 128), ("q2", 128))
            }
            if ch == 0:
                # startup DMA priority: wq, xt(chunk0), wv, trig(chunk0)
                nc.sync.dma_start(wpk[:, 0:3, :], wpk_d[:, 0:3, :])
                xt_next = issue_chunk_dma(0, split=True)
                nc.sync.dma_start(wpk[:, 3:6, :], wpk_d[:, 3:6, :])
                issue_chunk0_trig(xt_next)
            xt_c = xt_next
            if ch + 1 < NCH:
                xt_next = issue_chunk_dma(ch + 1, split=False)
            if ch == 0:
                nc.sync.dma_start(wpk[:, 6:9, :], wpk_d[:, 6:9, :])
                nc.sync.dma_start(msk[:], msk_d[:])
            # KV deferred from the previous chunk: runs stall-free at the
            # head of this chunk's tensor stream (its v/k deps are long done)
            kvsb_of = {}
            for g in carry_kv:
                kvsb_of[g] = issue_kv(g)
            issue_q(ch, xt_c, pair)
            issue_v(ch, xt_c)
            done = ch + 1
            # kn prefetch one chunk ahead of KV
            for g in range(GPD):
                if not kn_issued[g] and ready_chunk[g] - 1 <= done:
                    issue_kn_graph(g)
                    kn_issued[g] = True
            new_kv = [
                g for g in range(GPD) if not kv_issued[g] and ready_chunk[g] <= done
            ]
            for g in new_kv:
                kv_issued[g] = True
            if ch == NCH - 1:
                # no next chunk: run deferred KV now
                for g in new_kv:
                    kvsb_of[g] = issue_kv(g)
                new_kv = []
            issue_rope(ch, xt_c, pair)
            # out for graphs whose mb was issued >= 2 steps ago (rope slack);
            # on the final chunk, drain age-1 too so only the last graph's
            # out remains after the loop
            min_age = 1 if ch == NCH - 1 else 2
            for g, age in pending_out[:]:
                if age >= min_age:
                    issue_out(g, mbs_of[g])
                    pending_out.remove((g, age))
            pending_out = [(g, age + 1) for g, age in pending_out]
            for g in sorted(kvsb_of):
                mbs_of[g] = issue_mb(g, kvsb_of[g])
                pending_out.append((g, 1))
            carry_kv = new_kv
        for g, age in pending_out:
            issue_out(g, mbs_of[g])

    nc.compile()

    _CACHE[key] = (nc, NP)
    return nc, NP


last_exec_time_ns = None
last_results = None


def kernel(x, pos, batch, Wq, bq, Wv, bv, Wo, bo, freqs):
    global last_exec_time_ns
    x = np.asarray(x, dtype=np.float32)
    pos = np.asarray(pos, dtype=np.float32)
    batch = np.asarray(batch).astype(np.int64)
    Wq = np.asarray(Wq, dtype=np.float32)
    bq = np.asarray(bq, dtype=np.float32)
    Wv = np.asarray(Wv, dtype=np.float32)
    bv = np.asarray(bv, dtype=np.float32)
    Wo = np.asarray(Wo, dtype=np.float32)
    bo = np.asarray(bo, dtype=np.float32)
    freqs = np.asarray(freqs, dtype=np.float32)

    counts = np.bincount(batch, minlength=NUM_GRAPHS)
    starts = np.concatenate([[0], np.cumsum(counts)])
    has_bias = bool(np.any(bq) or np.any(bv))

    # sorted-deal assignment: rank graphs by size desc; core c, position j
    # gets graph order[j*NCORES + c]. slots[j] = max tile count at position j.
    order = np.argsort(-counts, kind="stable")
    assign = order.reshape(GPD, NCORES)  # [position, core] -> graph id
    # unrounded slots (multiple of 4 for DVE-friendly widths); boundary
    # k-tiles are duplicated+masked so KV stays per-graph exact
    slots = tuple(
        max(4, (int(counts[assign[j]].max()) + 3) // 4 * 4) for j in range(GPD)
    )
    soff = [0]
    for s_ in slots:
        soff.append(soff[-1] + s_)
    kti = [0]
    for j in range(GPD):
        cov = (soff[j] % 128 + slots[j] + 127) // 128
        kti.append(kti[-1] + cov)
    KTOT = kti[-1]

    nc, NP = _build(slots, has_bias)
    assert NP == (soff[-1] + 127) // 128 * 128

    WqA = Wq[:, _APERM]
    bf = ml_dtypes.bfloat16

    assert not has_bias, "packed-DMA path assumes zero biases"
    wos = Wo * (math.sqrt(2.0) / AVG)
    wpk = np.zeros((128, 9, E), dtype=bf)
    for bi in range(3):
        wpk[:, bi, :] = WqA[128 * bi : 128 * (bi + 1), :].astype(bf)
        wpk[:, 3 + bi, :] = Wv[128 * bi : 128 * (bi + 1), :].astype(bf)
        wpk[:, 6 + bi, :] = wos[128 * bi : 128 * (bi + 1), :].astype(bf)
    mskh = _kv_mask().astype(bf)

    # phase & trig on host (t = g*16+p, g-major)
    fr = freqs.reshape(NT, SD)
    phase = pos @ fr.T  # [N, 192] float32
    cphase = np.cos(phase)
    sphase = np.sin(phase)
    # k node-major, natural interleaved d-order (d = 2p+odd), then A-perm cols
    s2 = 1.0 / math.sqrt(2.0)
    kfull = np.empty((len(x), E), dtype=np.float32)
    k3 = kfull.reshape(len(x), G, D)
    ph3c = cphase.reshape(len(x), G, P)
    ph3s = sphase.reshape(len(x), G, P)
    k3[:, :, 0::2] = (ph3c - ph3s) * s2
    k3[:, :, 1::2] = (ph3c + ph3s) * s2
    kfullA = kfull[:, _APERM]

    in_maps = []
    for d in range(NCORES):
        xtt = np.zeros((128, 7, NP), dtype=bf)
        kn = np.zeros((KTOT * 128, E), dtype=bf)
        for lj in range(GPD):
            gb = int(assign[lj, d])
            s, e_, cnt = starts[gb], starts[gb + 1], counts[gb]
            if cnt == 0:
                continue
            o = soff[lj]
            xT = x[s:e_].T.astype(bf)  # [C, cnt]
            for bi in range(3):
                xtt[:, bi, o : o + cnt] = xT[128 * bi : 128 * (bi + 1)]
            xtt[:, 3, o : o + cnt] = cphase[s:e_, 0:128].T.astype(bf)
            xtt[:, 4, o : o + cnt] = sphase[s:e_, 0:128].T.astype(bf)
            chi = cphase[s:e_, 128:NT].T.astype(bf)
            shi = sphase[s:e_, 128:NT].T.astype(bf)
            xtt[0:64, 5, o : o + cnt] = chi
            xtt[64:128, 5, o : o + cnt] = shi
            xtt[0:64, 6, o : o + cnt] = shi
            xtt[64:128, 6, o : o + cnt] = chi
            ko = kti[lj] * 128 + (o % 128)
            kn[ko : ko + cnt, :] = kfullA[s:e_].astype(bf)
        in_maps.append({"xtt": xtt, "kn": kn, "wpk": wpk, "msk": mskh})

    want_trace = bool(int(os.environ.get("PLATCONV_TRACE", "0")))
    if want_trace:
        want_trace = _ensure_ntff_hook()
    res = run_bass_kernel_spmd(
        nc,
        in_maps,
        core_ids=list(range(NCORES)),
        trace=want_trace,
    )
    last_exec_time_ns = res.exec_time_ns
    global last_results
    last_results = res

    out = np.zeros((N, C), dtype=np.float32)
    for d in range(NCORES):
        ot = np.asarray(res.results[d]["outt"]).astype(np.float32).reshape(C, NP)
        for lj in range(GPD):
            gb = int(assign[lj, d])
            s, e_, cnt = starts[gb], starts[gb + 1], counts[gb]
            if cnt == 0:
                continue
            o = soff[lj]
            out[s:e_] = ot[:, o : o + cnt].T
    out += bo[None, :]
    return out


# revision 24
# speedup vs baseline: 1.0340x; 1.0340x over previous
"""Trainium2 Bass kernel v3 for nn_PlatonicConv (linear-attention GNN message passing).

Math (reference):
  q = rope(x@Wq + bq, phase);  k = rope(ones, phase);  v = x@Wv + bv
  KV_b[g] = (1/AVG) * sum_{n in graph b} k[n,g,:] (x) v[n,g,:]
  out[n]  = concat_g( q'[n,g,:] @ KV_b[g] ) @ Wo + bo

Device formulation (per core, 8 graphs/core, slot-padded columns):
  - host precomputes trig (feature-major cos/sin planes) and
    k = rope(ones) node-major with A-ORDERED columns.
  - A-order over rope pairs t = g*16+p (3 tiles of 128):
      rows   0:128 = E_t, t=0..127      (q0)
      rows 128:256 = O_t, t=0..127      (q1)
      rows 256:384 = E_t t=128..191 (64) | O_t t=128..191 (64)   (q2n)
  - x and trig ship together: one DRAM buffer [128, 7, NP] whose
    planes are 0:3 = x^T blocks, 3 = cos_low, 4 = sin_low,
    5 = [cos_high; sin_high], 6 = [sin_high; cos_high]. One DMA per
    chunk covers both the projections' input and the rope trig.
  - rope runs entirely on DVE with independent temps (no WAR chains);
    the 64-row q2 ops use plane 5/6 halves so every op has uniform
    input base partitions (out-shift only, as supported).
  - KV^T computed per graph into 2 psum regions with dense 4-group
    blocks; evacuated via block-diag MASK multiply (1 vector op each);
    Mb = KV^T-masked @ Wo via 5 contiguous-stationary matmuls.
  - out[n] = q'[n] @ M_{b(n)}; psum evacuated as bf16, DMA'd out.
  - startup: dummy 1x16 matmuls warm the PE (HAM un-throttle) while
    the first chunk's DMA lands; DMA issue order prioritizes
    wq -> xt(chunk0) -> wv -> trig(chunk0) so the pipeline fills fast.

Self-contained: hardcodes shapes; shards/pads on host inside kernel().
"""

import math
import os
from contextlib import ExitStack

import ml_dtypes
import numpy as np

import concourse.bacc as bacc_mod
import concourse.bass as bass
import concourse.mybir as mybir
import concourse.tile as tile
from concourse.bass_utils import run_bass_kernel_spmd


def _ensure_ntff_hook():
    """Register the axon NTFF profile hook if the image's antenv lacks it."""
    try:
        import antenv.axon_hooks  # noqa: F401

        return True
    except ImportError:
        pass
    try:
        import sys
        import types

        import antenv
        from trn_agent_boot.trn_boot import _ntff_profile_via_ctypes

        mod = types.ModuleType("antenv.axon_hooks")
        _hook = [None]
        mod.set_axon_ntff_profile_hook = lambda h: _hook.__setitem__(0, h)
        mod.get_axon_ntff_profile_hook = lambda: _hook[0]
        sys.modules["antenv.axon_hooks"] = mod
        antenv.axon_hooks = mod
        mod.set_axon_ntff_profile_hook(
            _ntff_profile_via_ctypes("/opt/axon/libaxon_pjrt.so")
        )
        return True
    except Exception:
        return False


FP32 = mybir.dt.float32
BF16 = mybir.dt.bfloat16
AF = mybir.ActivationFunctionType

N = 32768
C = 384
E = 384
G = 12
D = 32
P = 16
SD = 3
NUM_GRAPHS = 64
NCORES = 8
GPD = NUM_GRAPHS // NCORES
AVG = float(N) / NUM_GRAPHS
NT = 192  # rope pairs = G*P
W = 512  # streaming window
NWARM = int(os.environ.get("PLATCONV_NWARM", "64"))


def _a_order_cols():
    """perm such that A-order column r is original q-dim perm[r]."""
    perm = np.empty(E, dtype=np.int64)
    for r in range(E):
        if r < 128:
            t, odd = r, 0
        elif r < 256:
            t, odd = r - 128, 1
        elif r < 320:
            t, odd = 128 + (r - 256), 0
        else:
            t, odd = 128 + (r - 320), 1
        perm[r] = (t // 16) * 32 + 2 * (t % 16) + odd
    return perm


_APERM = _a_order_cols()


def _kv_mask():
    """[128, 640] bf16 mask selecting valid diag blocks of the KV psum layout.

    cols 0:256   = (e-chunk0, kA 0:256: E-main|O-main) -> g 0..3 rows
    cols 256:512 = (e-chunk1, kA 0:256)               -> g 4..7 rows
    cols 512:640 = (e-chunk2, kA 256:384: tail E|O)   -> g 8..11 rows
    """
    m = np.zeros((128, 640), dtype=np.float32)
    for ech in range(3):
        base = 256 * ech
        ncols = 256 if ech < 3 - 1 else 128
        for cc in range(ncols):
            if ech < 2:
                t = cc % 128  # E-main or O-main: t = 0..127
                g = t // 16
                ok = g - 4 * ech
            else:
                t = 128 + (cc % 64)  # tail: t = 128..191
                g = t // 16
                ok = g - 8
            if 0 <= ok < 4:
                m[32 * ok : 32 * ok + 32, base + cc] = 1.0
    return m


_CACHE = {}


def _build(slots: tuple, has_bias: bool):
    key = (slots, has_bias)
    if key in _CACHE:
        return _CACHE[key]

    assert len(slots) == GPD
    soff = [0]
    for s_ in slots:
        soff.append(soff[-1] + s_)
    NP = (soff[-1] + 127) // 128 * 128
    NTILE = NP // 128
    NCH = (NP + W - 1) // W
    # per-graph covering k-tile ranges (boundary tiles duplicated, host
    # masks each copy to its own graph's nodes)
    kti = [0]
    for j in range(GPD):
        cov = (soff[j] % 128 + slots[j] + 127) // 128
        kti.append(kti[-1] + cov)
    KTOT = kti[-1]

    nc = bacc_mod.Bacc()

    assert not has_bias, "packed-DMA path assumes zero biases"
    # planes 0:3 = x^T blocks; 3=cos_lo; 4=sin_lo; 5=[cos_hi;sin_hi];
    # 6=[sin_hi;cos_hi]
    xtt_d = nc.declare_dram_parameter("xtt", [128, 7, NP], BF16, isOutput=False)
    kn_d = nc.declare_dram_parameter("kn", [KTOT * 128, E], BF16, isOutput=False)
    # wpk rows: 0:3 wq blocks, 3:6 wv blocks, 6:9 wos blocks
    wpk_d = nc.declare_dram_parameter("wpk", [128, 9, E], BF16, isOutput=False)
    msk_d = nc.declare_dram_parameter("msk", [128, 640], BF16, isOutput=False)
    out_d = nc.declare_dram_parameter("outt", [3, 128, NP], BF16, isOutput=True)

    nk = 3

    with ExitStack() as ctx:
        tc = ctx.enter_context(tile.TileContext(nc))

        consts = ctx.enter_context(tc.tile_pool(name="consts", bufs=1))
        xtp = ctx.enter_context(tc.tile_pool(name="xtp", bufs=3))
        qsb = ctx.enter_context(tc.tile_pool(name="qsb", bufs=2))
        tmp = ctx.enter_context(tc.tile_pool(name="tmp", bufs=2))
        big = ctx.enter_context(tc.tile_pool(name="big", bufs=1))
        kvp = ctx.enter_context(tc.tile_pool(name="kvp", bufs=2))
        mbp = ctx.enter_context(tc.tile_pool(name="mbp", bufs=3))
        outp = ctx.enter_context(tc.tile_pool(name="outp", bufs=2))
        psum = ctx.enter_context(tc.tile_pool(name="psum", bufs=1, space="PSUM"))

        def pbank(tag):
            return psum.tile([128, W], FP32, tag=tag, name=tag)

        # ---- warmup: ACT table + dummy matmuls to un-throttle the PE ----
        warm = consts.tile([1, 16], BF16, tag="warm")
        nc.vector.memset(warm[:], 0.0)
        nc.scalar.activation(warm[:], warm[:], AF.Copy)
        wps = pbank("T5")
        for _ in range(NWARM):
            nc.tensor.matmul(
                wps[0:1, 0:16], warm[:, 0:1], warm[:, 0:16], start=True, stop=True
            )

        # ---- constants ----
        wpk = consts.tile([128, 9, E], BF16, tag="wpk")
        msk = consts.tile([128, 640], BF16, tag="msk")
        wq_t = [wpk[:, bi, :] for bi in range(3)]
        wv_t = [wpk[:, 3 + bi, :] for bi in range(3)]
        wos_t = [wpk[:, 6 + bi, :] for bi in range(3)]

        # ---- persistent SBUF ----
        q0 = big.tile([128, NP], BF16, tag="q0")
        q1 = big.tile([128, NP], BF16, tag="q1")
        q2n = big.tile([128, NP], BF16, tag="q2n")
        v_sb = big.tile([128, NTILE, E], BF16, tag="v_sb")
        k_sb = big.tile([128, KTOT, E], BF16, tag="k_sb")

        def issue_kn_graph(g):
            """Load masked k covering-tiles for graph g."""
            kview = kn_d[kti[g] * 128 : kti[g + 1] * 128, :].rearrange(
                "(t p) e -> p t e", p=128
            )
            nc.sync.dma_start(k_sb[:, kti[g] : kti[g + 1], :], kview)

        # PSUM plan:
        #  T0,T1,T2: Q proj (per chunk)
        #  T3: V proj even subtiles (cols 0:384) + KV tail region (cols 384:512)
        #  T4: V proj odd subtiles (cols 0:384) + KV main (cols 0:512)
        #  T5,T6,T7: Mb (qch0..2), then out windows rotate T5..T7

        # ------------------------------------------------------------------
        # builders
        # ------------------------------------------------------------------

        def wl(ch):
            return min(W, NP - ch * W)

        def issue_chunk_dma(ch, split):
            n0 = ch * W
            wlen = wl(ch)
            t = xtp.tile([128, 7, W], BF16, tag="xtc", name="xtc")
            if split:
                nc.sync.dma_start(t[:, 0:3, :wlen], xtt_d[:, 0:3, n0 : n0 + wlen])
            else:
                nc.sync.dma_start(t[:, :, :wlen], xtt_d[:, :, n0 : n0 + wlen])
            return t

        def issue_chunk0_trig(t):
            nc.sync.dma_start(t[:, 3:7, : wl(0)], xtt_d[:, 3:7, 0 : wl(0)])

        def issue_q(ch, t, pair):
            """Q projection matmuls + psum evac for chunk ch."""
            wlen = wl(ch)
            qb = [pbank("T0"), pbank("T1"), pbank("T2")]
            for j, ps in enumerate(qb):
                c0 = 128 * j
                for ki in range(nk):
                    nc.tensor.matmul(
                        ps[:, :wlen],
                        wq_t[ki][:, c0 : c0 + 128],
                        t[:, ki, :wlen],
                        start=(ki == 0),
                        stop=(ki == nk - 1),
                    )
            nc.scalar.activation(pair["qe"][:, :wlen], qb[0][:, :wlen], AF.Copy)
            nc.scalar.activation(pair["qo"][:, :wlen], qb[1][:, :wlen], AF.Copy)
            nc.scalar.activation(pair["q2"][:, :wlen], qb[2][:, :wlen], AF.Copy)

        def issue_v(ch, t):
            """V projection: 4 node subtiles, banks T3/T4 alternating."""
            for sub in range(wl(ch) // 128):
                ti = ch * (W // 128) + sub
                f0 = sub * 128
                vps = pbank("T3" if sub % 2 == 0 else "T4")
                for ki in range(nk):
                    nc.tensor.matmul(
                        vps[:, 0:E],
                        t[:, ki, f0 : f0 + 128],
                        wv_t[ki][:],
                        start=(ki == 0),
                        stop=(ki == nk - 1),
                    )
                nc.scalar.activation(v_sb[:, ti, :], vps[:, 0:E], AF.Copy)

        def issue_rope(ch, t, pair):
            """Rope for chunk ch, all on DVE with independent temps."""
            wlen = wl(ch)
            csl = slice(ch * W, ch * W + wlen)
            sl = slice(0, wlen)
            qe, qo, q2 = pair["qe"], pair["qo"], pair["q2"]
            clf = t[:, 3, sl]
            slf = t[:, 4, sl]
            ta0 = tmp.tile([128, W], BF16, tag="ta0", name="ta0")
            tb0 = tmp.tile([128, W], BF16, tag="tb0", name="tb0")
            ta1 = tmp.tile([128, W], BF16, tag="ta1", name="ta1")
            tb1 = tmp.tile([128, W], BF16, tag="tb1", name="tb1")
            tg1 = tmp.tile([64, W], BF16, tag="tg1", name="tg1")
            tg2 = tmp.tile([64, W], BF16, tag="tg2", name="tg2")
            tc2 = tmp.tile([64, W], BF16, tag="tc2", name="tc2")
            td2 = tmp.tile([64, W], BF16, tag="td2", name="td2")
            nc.vector.tensor_mul(ta0[:, sl], qe[:, sl], clf)
            nc.vector.tensor_mul(tb0[:, sl], qo[:, sl], slf)
            nc.vector.tensor_mul(ta1[:, sl], qe[:, sl], slf)
            nc.vector.tensor_mul(tb1[:, sl], qo[:, sl], clf)
            # plane 5 = [cos_hi; sin_hi], plane 6 = [sin_hi; cos_hi]
            nc.vector.tensor_mul(tg1[:, sl], q2[0:64, sl], t[0:64, 5, sl])
            nc.vector.tensor_mul(tg2[:, sl], q2[64:128, sl], t[64:128, 5, sl])
            nc.vector.tensor_mul(tc2[:, sl], q2[0:64, sl], t[0:64, 6, sl])
            nc.vector.tensor_mul(td2[:, sl], q2[64:128, sl], t[64:128, 6, sl])
            nc.vector.tensor_sub(q0[:, csl], ta0[:, sl], tb0[:, sl])
            nc.vector.tensor_add(q1[:, csl], ta1[:, sl], tb1[:, sl])
            nc.vector.tensor_sub(q2n[0:64, csl], tg1[:, sl], tg2[:, sl])
            nc.vector.tensor_add(q2n[64:128, csl], tc2[:, sl], td2[:, sl])

        def issue_kv(j):
            """KV matmuls + mask evac for graph j. Returns kvsb tile."""
            t0 = soff[j] // 128
            TPS = kti[j + 1] - kti[j]
            kvm = pbank("T4")  # main: (e0,kA 0:256) at 0:256, (e1,kA 0:256) at 256:512
            kvt = pbank("T3")  # tail: (e2, kA 256:384) at cols 384:512
            # region-major: start=True clears the whole bank's has_written
            # marks, so each region's accumulation must complete before the
            # next region in the same bank begins.
            kvsb = kvp.tile([128, 640], BF16, tag="kvsb", name="kvsb")
            regions = (
                (kvm[:, 0:256], slice(0, 128), slice(0, 256)),
                (kvm[:, 256:512], slice(128, 256), slice(0, 256)),
                (kvt[:, 384:512], slice(256, 384), slice(256, 384)),
            )
            for dst, vsl, ksl in regions:
                for tt in range(TPS):
                    nc.tensor.matmul(
                        dst,
                        v_sb[:, t0 + tt, vsl],
                        k_sb[:, kti[j] + tt, ksl],
                        start=(tt == 0),
                        stop=(tt == TPS - 1),
                    )
            nc.vector.tensor_mul(kvsb[:, 0:512], kvm[:], msk[:, 0:512])
            nc.vector.tensor_mul(kvsb[:, 512:640], kvt[:, 384:512], msk[:, 512:640])
            return kvsb

        def issue_mb(j, kvsb):
            # Mb: 5 matmuls, contiguous stationaries. Uses the Q banks
            # (T0-2, evacuated early in the chunk) so Mb never waits on
            # the out-window evacuations that own T5-7.
            mb_ps = [pbank("T0"), pbank("T1"), pbank("T2")]
            groups = (
                ((kvsb[:, 0:128], wos_t[0]), (kvsb[:, 256:384], wos_t[1])),
                ((kvsb[:, 128:256], wos_t[0]), (kvsb[:, 384:512], wos_t[1])),
                ((kvsb[:, 512:640], wos_t[2]),),
            )
            mbs = []
            for i, grp in enumerate(groups):
                for gi, (stat, mov) in enumerate(grp):
                    nc.tensor.matmul(
                        mb_ps[i][:, 0:C], stat, mov[:],
                        start=(gi == 0), stop=(gi == len(grp) - 1),
                    )
                mb = mbp.tile([128, C], BF16, tag=f"mb{i}", name=f"mb{i}")
                nc.scalar.activation(mb[:], mb_ps[i][:, 0:C], AF.Copy)
                mbs.append(mb)
            return mbs

        oti = [0]

        def issue_out(j, mbs):
            """Out matmuls + evac + DMA for graph j. Windows split evenly
            (e.g. 572 -> 288+284, not 512+60) to avoid tiny-N matmuls."""
            slot0 = soff[j]
            nwin = (slots[j] + W - 1) // W
            base = slots[j] // nwin // 4 * 4
            wins = []
            o = 0
            for i in range(nwin):
                w = slots[j] - o - base * (nwin - 1 - i)
                wins.append((slot0 + o, w))
                o += w
            for w0, w in wins:
                ost = outp.tile(
                    [128, 3, W], BF16, tag=f"ost{oti[0] % 2}", name=f"ost{oti[0] % 2}"
                )
                for cch in range(3):
                    cc = slice(128 * cch, 128 * (cch + 1))
                    ot = pbank(f"T{5 + cch}")
                    nc.tensor.matmul(
                        ot[:, :w], mbs[0][:, cc], q0[:, w0 : w0 + w],
                        start=True, stop=False,
                    )
                    nc.tensor.matmul(
                        ot[:, :w], mbs[1][:, cc], q1[:, w0 : w0 + w],
                        start=False, stop=False,
                    )
                    nc.tensor.matmul(
                        ot[:, :w], mbs[2][:, cc], q2n[:, w0 : w0 + w],
                        start=False, stop=True,
                    )
                    if cch == 1:
                        nc.scalar.activation(ost[:, cch, :w], ot[:, :w], AF.Copy)
                    else:
                        nc.vector.tensor_copy(ost[:, cch, :w], ot[:, :w])
                nc.sync.dma_start(
                    out_d[:, :, w0 : w0 + w].rearrange("c p w -> p c w"),
                    ost[:, :, :w],
                )
                oti[0] += 1

        # ------------------------------------------------------------------
        # interleaved schedule
        # ------------------------------------------------------------------
        ready_chunk = [
            (soff[g + 1] + W - 1) // W for g in range(GPD)
        ]  # chunks needed before KV_g / OUT_g
        kv_issued = [False] * GPD
        kn_issued = [False] * GPD
        mbs_of = {}
        pending_out = []
        carry_kv = []  # graphs whose KV is deferred to next chunk's stream

        xt_next = None
        for ch in range(NCH):
            pair = {
                k_: qsb.tile([p_, W], BF16, tag=k_, name=k_)
                for k_, p_ in (("qe", 128), ("qo", 128), ("q2", 128))
            }
            if ch == 0:
                # startup DMA priority: wq, xt(chunk0), wv, trig(chunk0)
                nc.sync.dma_start(wpk[:, 0:3, :], wpk_d[:, 0:3, :])
                xt_next = issue_chunk_dma(0, split=True)
                nc.sync.dma_start(wpk[:, 3:6, :], wpk_d[:, 3:6, :])
                issue_chunk0_trig(xt_next)
            xt_c = xt_next
            if ch + 1 < NCH:
                xt_next = issue_chunk_dma(ch + 1, split=False)
            if ch == 0:
                nc.sync.dma_start(wpk[:, 6:9, :], wpk_d[:, 6:9, :])
                nc.sync.dma_start(msk[:], msk_d[:])
            # KV deferred from the previous chunk: runs stall-free at the
            # head of this chunk's tensor stream (its v/k deps are long done)
            kvsb_of = {}
            for g in carry_kv:
                kvsb_of[g] = issue_kv(g)
            issue_q(ch, xt_c, pair)
            issue_v(ch, xt_c)
            done = ch + 1
            # kn prefetch one chunk ahead of KV
            for g in range(GPD):
                if not kn_issued[g] and ready_chunk[g] - 1 <= done:
                    issue_kn_graph(g)
                    kn_issued[g] = True
            new_kv = [
                g for g in range(GPD) if not kv_issued[g] and ready_chunk[g] <= done
            ]
            for g in new_kv:
                kv_issued[g] = True
            if ch == NCH - 1:
                # no next chunk: run deferred KV now
                for g in new_kv:
                    kvsb_of[g] = issue_kv(g)
                new_kv = []
            issue_rope(ch, xt_c, pair)
            # out for graphs whose mb was issued >= 2 steps ago (rope slack);
            # on the final chunk, drain age-1 too so only the last graph's
            # out remains after the loop
            min_age = 1 if ch == NCH - 1 else 2
            for g, age in pending_out[:]:
                if age >= min_age:
                    issue_out(g, mbs_of[g])
                    pending_out.remove((g, age))
            pending_out = [(g, age + 1) for g, age in pending_out]
            for g in sorted(kvsb_of):
                mbs_of[g] = issue_mb(g, kvsb_of[g])
                pending_out.append((g, 1))
            carry_kv = new_kv
        for g, age in pending_out:
            issue_out(g, mbs_of[g])

    nc.compile()

    _CACHE[key] = (nc, NP)
    return nc, NP


last_exec_time_ns = None
last_results = None


def kernel(x, pos, batch, Wq, bq, Wv, bv, Wo, bo, freqs):
    global last_exec_time_ns
    x = np.asarray(x, dtype=np.float32)
    pos = np.asarray(pos, dtype=np.float32)
    batch = np.asarray(batch).astype(np.int64)
    Wq = np.asarray(Wq, dtype=np.float32)
    bq = np.asarray(bq, dtype=np.float32)
    Wv = np.asarray(Wv, dtype=np.float32)
    bv = np.asarray(bv, dtype=np.float32)
    Wo = np.asarray(Wo, dtype=np.float32)
    bo = np.asarray(bo, dtype=np.float32)
    freqs = np.asarray(freqs, dtype=np.float32)

    counts = np.bincount(batch, minlength=NUM_GRAPHS)
    starts = np.concatenate([[0], np.cumsum(counts)])
    has_bias = bool(np.any(bq) or np.any(bv))

    # sorted-deal assignment: rank graphs by size desc; core c, position j
    # gets graph order[j*NCORES + c]. slots[j] = max tile count at position j.
    order = np.argsort(-counts, kind="stable")
    assign = order.reshape(GPD, NCORES)  # [position, core] -> graph id
    # unrounded slots (multiple of 4 for DVE-friendly widths); boundary
    # k-tiles are duplicated+masked so KV stays per-graph exact
    slots = tuple(
        max(4, (int(counts[assign[j]].max()) + 3) // 4 * 4) for j in range(GPD)
    )
    soff = [0]
    for s_ in slots:
        soff.append(soff[-1] + s_)
    kti = [0]
    for j in range(GPD):
        cov = (soff[j] % 128 + slots[j] + 127) // 128
        kti.append(kti[-1] + cov)
    KTOT = kti[-1]

    nc, NP = _build(slots, has_bias)
    assert NP == (soff[-1] + 127) // 128 * 128

    WqA = Wq[:, _APERM]
    bf = ml_dtypes.bfloat16

    assert not has_bias, "packed-DMA path assumes zero biases"
    wos = Wo * (math.sqrt(2.0) / AVG)
    wpk = np.zeros((128, 9, E), dtype=bf)
    for bi in range(3):
        wpk[:, bi, :] = WqA[128 * bi : 128 * (bi + 1), :].astype(bf)
        wpk[:, 3 + bi, :] = Wv[128 * bi : 128 * (bi + 1), :].astype(bf)
        wpk[:, 6 + bi, :] = wos[128 * bi : 128 * (bi + 1), :].astype(bf)
    mskh = _kv_mask().astype(bf)

    # phase & trig on host (t = g*16+p, g-major)
    fr = freqs.reshape(NT, SD)
    phase = pos @ fr.T  # [N, 192] float32
    cphase = np.cos(phase)
    sphase = np.sin(phase)
    # k node-major, natural interleaved d-order (d = 2p+odd), then A-perm cols
    s2 = 1.0 / math.sqrt(2.0)
    kfull = np.empty((len(x), E), dtype=np.float32)
    k3 = kfull.reshape(len(x), G, D)
    ph3c = cphase.reshape(len(x), G, P)
    ph3s = sphase.reshape(len(x), G, P)
    k3[:, :, 0::2] = (ph3c - ph3s) * s2
    k3[:, :, 1::2] = (ph3c + ph3s) * s2
    kfullA = kfull[:, _APERM]

    in_maps = []
    for d in range(NCORES):
        xtt = np.zeros((128, 7, NP), dtype=bf)
        kn = np.zeros((KTOT * 128, E), dtype=bf)
        for lj in range(GPD):
            gb = int(assign[lj, d])
            s, e_, cnt = starts[gb], starts[gb + 1], counts[gb]
            if cnt == 0:
                continue
            o = soff[lj]
            xT = x[s:e_].T.astype(bf)  # [C, cnt]
            for bi in range(3):
                xtt[:, bi, o : o + cnt] = xT[128 * bi : 128 * (bi + 1)]
            xtt[:, 3, o : o + cnt] = cphase[s:e_, 0:128].T.astype(bf)
            xtt[:, 4, o : o + cnt] = sphase[s:e_, 0:128].T.astype(bf)
            chi = cphase[s:e_, 128:NT].T.astype(bf)
            shi = sphase[s:e_, 128:NT].T.astype(bf)
            xtt[0:64, 5, o : o + cnt] = chi
            xtt[64:128, 5, o : o + cnt] = shi
            xtt[0:64, 6, o : o + cnt] = shi
            xtt[64:128, 6, o : o + cnt] = chi
            ko = kti[lj] * 128 + (o % 128)
            kn[ko : ko + cnt, :] = kfullA[s:e_].astype(bf)
        in_maps.append({"xtt": xtt, "kn": kn, "wpk": wpk, "msk": mskh})

    want_trace = bool(int(os.environ.get("PLATCONV_TRACE", "0")))
    if want_trace:
        want_trace = _ensure_ntff_hook()
    res = run_bass_kernel_spmd(
        nc,
        in_maps,
        core_ids=list(range(NCORES)),
        trace=want_trace,
    )
    last_exec_time_ns = res.exec_time_ns
    global last_results
    last_results = res

    out = np.zeros((N, C), dtype=np.float32)
    for d in range(NCORES):
        ot = np.asarray(res.results[d]["outt"]).astype(np.float32).reshape(C, NP)
        for lj in range(GPD):
            gb = int(assign[lj, d])
            s, e_, cnt = starts[gb], starts[gb + 1], counts[gb]
            if cnt == 0:
                continue
            o = soff[lj]
            out[s:e_] = ot[:, o : o + cnt].T
    out += bo[None, :]
    return out
